# revision 9
# baseline (speedup 1.0000x reference)
"""Channel-attention block (q/k/v 1x1-conv+BN+ReLU, attention over channels
contracted over N points, softmax, weighted sum) for B=8, C=128, N=20000.

Sharding: data-parallel over B — one sample per NeuronCore, 8 cores, no
cross-core communication. Inside each core:

  phase 1 (streaming over N in 512-point tiles / 128-point chunks):
    - v = relu(Wv' @ pc + bv')      natural layout [C, n], fp16, kept in SBUF
    - qk^T chunk = pc_chunk^T @ [Wq'^T | Wk'^T]   (one fp32r matmul, N=256)
    - +bias (DVE, replicated-bias tile), relu+cast->fp16 (GPSIMD)
    - attn accumulation (fp16 matmul): [qs·k^T | qs·[x^T|1]] into one
      persistent PSUM bank ([128, 133]); graph-context never materialized.
  epilogue:
    logits = S1 + S2ext @ Wgext^T (rank-5 correction), softmax via
    (-max bias) exp with fused row-sum; normalization deferred to phase 2.
  phase 2:
    out = attn^T-stationary fp16 matmul over v tiles (N=512), PSUM->SBUF
    copy applies the 1/rowsum scale, DMA to DRAM.

All BN/temperature folding is done on the host (weights only, O(C^2)).
"""

import numpy as np

import concourse.bass as bass
import concourse.bacc as bacc
import concourse.tile as tile
from concourse import mybir
from concourse.bass_utils import run_bass_kernel_spmd

F32 = mybir.dt.float32
F32R = mybir.dt.float32r
F16 = mybir.dt.float16

B, C, N = 8, 128, 20000
EPS = 1e-5
TS = 512                     # n-tile size (v matmul free dim / PSUM bank)
CH = 128                     # chunk size (attn contraction partition dim)
TILES = [(i * TS, TS) for i in range(N // TS)]
if N % TS:
    TILES.append((N - N % TS, N % TS))

# chunks where the qk bias is applied by a rank-1 PE matmul (ACT does the
# relu straight from PSUM) instead of DVE-add + GPSIMD-relu. num/den split.
PE_BIAS_NUM = 0
PE_BIAS_DEN = 4

LAST_EXEC_NS = None
LAST_RESULTS = None

_CACHE = {}

_AXON_SO = "/opt/axon/libaxon_pjrt.so"


def _setup_profiling():
    """Make run_bass_kernel_spmd(trace=True) work in this container: inject
    the antenv.axon_hooks module (absent in the agent image) with the
    ctypes NTFF hook, and keep profile artifacts local."""
    import sys
    import types

    import concourse.bass_utils as bu

    if not getattr(bu, "_local_artifacts", False):
        bu.upload_artifacts = lambda tmpdir: tmpdir
        bu._local_artifacts = True

    if "antenv.axon_hooks" not in sys.modules:
        mod = types.ModuleType("antenv.axon_hooks")
        mod._hook = None

        def set_axon_ntff_profile_hook(h):
            mod._hook = h

        def get_axon_ntff_profile_hook():
            return mod._hook

        mod.set_axon_ntff_profile_hook = set_axon_ntff_profile_hook
        mod.get_axon_ntff_profile_hook = get_axon_ntff_profile_hook
        sys.modules["antenv.axon_hooks"] = mod
        import antenv

        antenv.axon_hooks = mod
        try:
            from trn_agent_boot.trn_boot import _ntff_profile_via_ctypes

            mod._hook = _ntff_profile_via_ctypes(_AXON_SO)
        except Exception as e:  # profiling degrades, run still works
            print(f"NTFF hook install failed: {e}")


def _build_bass():
    nc = bacc.Bacc("TRN2", target_bir_lowering=False)

    pc_d = nc.declare_dram_parameter("pc", [C, N], F32R, isOutput=False)
    xt_d = nc.declare_dram_parameter("xt5", [N, 5], F16, isOutput=False)
    wqk_d = nc.declare_dram_parameter("wqk", [C, 2 * C], F32R, isOutput=False)
    wv_d = nc.declare_dram_parameter("wvT", [C, C], F32R, isOutput=False)
    brep_d = nc.declare_dram_parameter("brep", [C, 2 * C], F32, isOutput=False)
    bv_d = nc.declare_dram_parameter("bv", [C, 1], F32, isOutput=False)
    wgt_d = nc.declare_dram_parameter("wgt", [5, C], F32R, isOutput=False)
    idf_d = nc.declare_dram_parameter("idf32", [C, C], F32, isOutput=False)
    idh_d = nc.declare_dram_parameter("idf16", [C, C], F16, isOutput=False)
    out_d = nc.declare_dram_parameter("out", [C, N], F32, isOutput=True)

    with tile.TileContext(nc) as tc:
        with (
            tc.tile_pool(name="persist", bufs=1) as persist,
            tc.tile_pool(name="consts", bufs=1) as consts,
            tc.tile_pool(name="attn_ps", bufs=1, space="PSUM") as attn_pool,
        ):
            # constants
            wqk_s = consts.tile([C, 2 * C], F32R)
            nc.sync.dma_start(out=wqk_s, in_=wqk_d[:, :])
            wv_s = consts.tile([C, C], F32R)
            nc.sync.dma_start(out=wv_s, in_=wv_d[:, :])
            brep_s = consts.tile([C, 2 * C], F32)
            nc.sync.dma_start(out=brep_s, in_=brep_d[:, :])
            bv_s = consts.tile([C, 1], F32)
            nc.sync.dma_start(out=bv_s, in_=bv_d[:, :])
            wgt_s = consts.tile([5, C], F32R)
            nc.sync.dma_start(out=wgt_s, in_=wgt_d[:, :])
            idf_s = consts.tile([C, C], F32)
            nc.sync.dma_start(out=idf_s, in_=idf_d[:, :])
            idh_s = consts.tile([C, C], F16)
            nc.sync.dma_start(out=idh_s, in_=idh_d[:, :])
            if PE_BIAS_NUM > 0:
                ones_s = consts.tile([1, CH], F32R)
                nc.vector.memset(ones_s, 1.0)
                brow_s = consts.tile([1, 2 * C], F32R)
                nc.sync.dma_start(out=brow_s, in_=brep_d[0:1, :].bitcast(F32R))

            # v, resident in SBUF for the whole kernel (fp16, ~39KB/partition)
            v_s = persist.tile([C, N], F16)

            # attn logits accumulator: [c, d(128) | j(5)] — one PSUM bank
            attn_ps = attn_pool.tile([C, C + 5], F32)

            gchunk = 0
            nchunks = sum((ts + CH - 1) // CH for _, ts in TILES)

            with (
                tc.tile_pool(name="pc", bufs=3) as pc_pool,
                tc.tile_pool(name="qkb", bufs=3) as qkb_pool,
                tc.tile_pool(name="comb", bufs=3) as comb_pool,
                tc.tile_pool(name="qk_ps", bufs=3, space="PSUM") as qk_ps_pool,
                tc.tile_pool(name="v_ps", bufs=2, space="PSUM") as v_ps_pool,
            ):
                for n0, ts in TILES:
                    pc_t = pc_pool.tile([C, TS], F32R, tag="pc")
                    nc.sync.dma_start(out=pc_t[:, :ts], in_=pc_d[:, n0:n0 + ts])

                    # v tile, natural layout
                    v_ps = v_ps_pool.tile([C, TS], F32, tag="vps")
                    nc.tensor.matmul(
                        v_ps[:, :ts], lhsT=wv_s, rhs=pc_t[:, :ts],
                        start=True, stop=True,
                    )
                    nc.scalar.activation(
                        v_s[:, n0:n0 + ts], v_ps[:, :ts],
                        mybir.ActivationFunctionType.Relu,
                        bias=bv_s, scale=1.0,
                    )

                    for c0 in range(0, ts, CH):
                        nt = min(CH, ts - c0)
                        pe_bias = (gchunk % PE_BIAS_DEN) < PE_BIAS_NUM

                        qk_ps = qk_ps_pool.tile([CH, 2 * C], F32, tag="qkps")
                        nc.tensor.matmul(
                            qk_ps[:nt, :],
                            lhsT=pc_t[:, c0:c0 + nt],
                            rhs=wqk_s,
                            start=True, stop=not pe_bias,
                        )
                        comb = comb_pool.tile([CH, 2 * C + 5], F16, tag="comb")
                        if pe_bias:
                            nc.tensor.matmul(
                                qk_ps[:nt, :],
                                lhsT=ones_s[:, :nt],
                                rhs=brow_s,
                                start=False, stop=True,
                            )
                            nc.scalar.activation(
                                comb[:nt, 0:2 * C], qk_ps[:nt, :],
                                mybir.ActivationFunctionType.Relu,
                                bias=0.0, scale=1.0,
                            )
                        else:
                            qkb = qkb_pool.tile([CH, 2 * C], F32, tag="qkb")
                            nc.vector.tensor_add(
                                qkb[:nt, :], qk_ps[:nt, :], brep_s[:nt, :]
                            )
                            nc.gpsimd.tensor_scalar_max(
                                comb[:nt, 0:2 * C], qkb[:nt, :], 0.0
                            )
                        nc.sync.dma_start(
                            out=comb[:nt, 2 * C:2 * C + 5],
                            in_=xt_d[n0 + c0:n0 + c0 + nt, :],
                        )
                        nc.tensor.matmul(
                            attn_ps,
                            lhsT=comb[:nt, 0:C],
                            rhs=comb[:nt, C:2 * C + 5],
                            start=(gchunk == 0), stop=False,
                            skip_group_check=True,
                        )
                        gchunk += 1
                assert gchunk == nchunks

            # ---- epilogue: graph-context correction + softmax ----
            with (
                tc.tile_pool(name="epi", bufs=1) as epi,
                tc.tile_pool(name="epi_ps", bufs=1, space="PSUM") as epi_ps,
            ):
                s2_s = epi.tile([C, 5], F32)
                nc.scalar.copy(s2_s, attn_ps[:, C:C + 5])
                s2t_ps = epi_ps.tile([5, C], F32, tag="s2t")
                nc.tensor.transpose(s2t_ps, s2_s, idf_s)
                s2t_s = epi.tile([5, C], F32R)
                nc.scalar.copy(s2t_s, s2t_ps)
                nc.tensor.matmul(
                    attn_ps[:, 0:C],
                    lhsT=s2t_s,
                    rhs=wgt_s,
                    start=False, stop=True,
                    skip_group_check=True,
                )

                mx = epi.tile([C, 1], F32)
                nc.vector.tensor_reduce(
                    mx, attn_ps[:, 0:C],
                    axis=mybir.AxisListType.X, op=mybir.AluOpType.max,
                )
                negmx = epi.tile([C, 1], F32)
                nc.vector.tensor_scalar_mul(negmx, mx, -1.0)
                p_s = epi.tile([C, C], F16)
                denom = epi.tile([C, 1], F32)
                nc.scalar.activation(
                    p_s, attn_ps[:, 0:C],
                    mybir.ActivationFunctionType.Exp,
                    bias=negmx, scale=1.0, accum_out=denom,
                )
                inv = persist.tile([C, 1], F32)
                nc.vector.reciprocal(inv, denom)

                attnT_ps = epi_ps.tile([C, C], F16, tag="attnT")
                nc.tensor.transpose(attnT_ps, p_s, idh_s)
                attnT_s = persist.tile([C, C], F16)
                nc.vector.tensor_copy(attnT_s, attnT_ps)

            # ---- phase 2: out = (1/denom) * attn_exp @ v ----
            with (
                tc.tile_pool(name="osb", bufs=3) as o_pool,
                tc.tile_pool(name="o_ps", bufs=2, space="PSUM") as o_ps_pool,
            ):
                for n0, ts in TILES:
                    o_ps = o_ps_pool.tile([C, TS], F32, tag="ops")
                    nc.tensor.matmul(
                        o_ps[:, :ts],
                        lhsT=attnT_s,
                        rhs=v_s[:, n0:n0 + ts],
                        start=True, stop=True,
                    )
                    o_s = o_pool.tile([C, TS], F32, tag="osb")
                    nc.scalar.activation(
                        o_s[:, :ts], o_ps[:, :ts],
                        mybir.ActivationFunctionType.Copy,
                        bias=0.0, scale=inv,
                    )
                    nc.sync.dma_start(out=out_d[:, n0:n0 + ts], in_=o_s[:, :ts])

    nc.finalize()
    return nc


def _prep_host(inputs):
    """Fold BN + temperature into weights on the host. O(C^2) work only."""
    f = np.float32
    pc = np.ascontiguousarray(np.asarray(inputs["PointCN1"], f)[..., 0])  # [B,C,N]
    x = np.asarray(inputs["x"], f)[..., 0]                                # [B,4,N]

    def fold(W, g, b, m, v, temp=1.0):
        # y*scale + (b - m*scale), then /temp, all folded into W and shift
        s = (np.asarray(g, f) / np.sqrt(np.asarray(v, f) + EPS)) / temp
        Wp = np.asarray(W, f) * s[:, None]
        sh = (np.asarray(b, f) / temp) - np.asarray(m, f) * s
        return Wp, sh

    temp = np.sqrt(np.float32(C))
    Wq, sq = fold(inputs["Wq"], inputs["gq"], inputs["bq"], inputs["mq"],
                  inputs["vq"], temp)
    Wk, sk = fold(inputs["Wk"], inputs["gk"], inputs["bk"], inputs["mk"],
                  inputs["vk"])
    Wv, sv = fold(inputs["Wv"], inputs["gv"], inputs["bv"], inputs["mv"],
                  inputs["vv"])

    wqk = np.ascontiguousarray(np.concatenate([Wq.T, Wk.T], axis=1))  # [C,256]
    wvT = np.ascontiguousarray(np.asarray(Wv, f).T)                   # [C,C]
    brep = np.ascontiguousarray(
        np.tile(np.concatenate([sq, sk])[None, :], (C, 1)).astype(f))  # [C,256]
    bv = np.ascontiguousarray(sv[:, None])                            # [C,1]

    Wg1 = np.asarray(inputs["Wg1"], f)  # [C,2]
    Wg2 = np.asarray(inputs["Wg2"], f)  # [C,2]
    bg = np.asarray(inputs["bg1"], f) + np.asarray(inputs["bg2"], f)  # [C]
    wgext = np.concatenate([Wg1, Wg2, bg[:, None]], axis=1)           # [C,5]
    wgt = np.ascontiguousarray(wgext.T)                               # [5,C]

    xt5 = np.empty((B, N, 5), np.float16)
    for b in range(B):
        xt5[b, :, 0:4] = x[b].T.astype(np.float16)
        xt5[b, :, 4] = np.float16(1.0)

    idf32 = np.eye(C, dtype=np.float32)
    idf16 = np.eye(C, dtype=np.float16)

    in_maps = []
    for b in range(B):
        in_maps.append({
            "pc": pc[b],
            "xt5": np.ascontiguousarray(xt5[b]),
            "wqk": wqk,
            "wvT": wvT,
            "brep": brep,
            "bv": bv,
            "wgt": wgt,
            "idf32": idf32,
            "idf16": idf16,
        })
    return in_maps


def kernel(profile=False, **inputs):
    global LAST_EXEC_NS, LAST_RESULTS
    if profile:
        _setup_profiling()
    if "nc" not in _CACHE:
        _CACHE["nc"] = _build_bass()
    nc = _CACHE["nc"]
    in_maps = _prep_host(inputs)
    res = run_bass_kernel_spmd(
        nc, in_maps, core_ids=list(range(B)), trace=bool(profile),
    )
    LAST_EXEC_NS = res.exec_time_ns
    LAST_RESULTS = res
    out = np.stack([res.results[b]["out"] for b in range(B)], axis=0)
    return out[..., None].astype(np.float32)


# revision 12
# speedup vs baseline: 3.4739x; 3.4739x over previous
"""Channel-attention block (q/k/v 1x1-conv+BN+ReLU, attention over channels
contracted over N points, softmax, weighted sum) for B=8, C=128, N=20000.

Sharding: data-parallel over B — one sample per NeuronCore, 8 cores, no
cross-core communication. Inside each core:

  phase 1 (streaming over N in 512-point tiles / 128-point chunks):
    - v = relu(Wv' @ pc + bv')      natural layout [C, n], fp16, kept in SBUF
    - qk^T chunk = pc_chunk^T @ [Wq'^T | Wk'^T]   (one fp32r matmul, N=256)
    - +bias (DVE, replicated-bias tile), relu+cast->fp16 (GPSIMD)
    - attn accumulation (fp16 matmul): [qs·k^T | qs·[x^T|1]] into one
      persistent PSUM bank ([128, 133]); graph-context never materialized.
  epilogue:
    logits = S1 + S2ext @ Wgext^T (rank-5 correction), softmax via
    (-max bias) exp with fused row-sum; normalization deferred to phase 2.
  phase 2:
    out = attn^T-stationary fp16 matmul over v tiles (N=512), PSUM->SBUF
    copy applies the 1/rowsum scale, DMA to DRAM.

All BN/temperature folding is done on the host (weights only, O(C^2)).
"""

import numpy as np

import concourse.bass as bass
import concourse.bacc as bacc
import concourse.tile as tile
from concourse import mybir
from concourse.bass_utils import run_bass_kernel_spmd

F32 = mybir.dt.float32
F32R = mybir.dt.float32r
F16 = mybir.dt.float16

B, C, N = 8, 128, 20000
EPS = 1e-5
TS = 512                     # n-tile size (v matmul free dim / PSUM bank)
CH = 128                     # chunk size (attn contraction partition dim)
TILES = [(i * TS, TS) for i in range(N // TS)]
if N % TS:
    TILES.append((N - N % TS, N % TS))
NCH = sum((ts + CH - 1) // CH for _, ts in TILES)   # 157 chunks

# chunks where the qk bias is applied by a rank-1 PE matmul (ACT does the
# relu straight from PSUM) instead of the fused DVE relu-add. num/den split.
PE_BIAS_NUM = 0
PE_BIAS_DEN = 4

_RELU_ADD = {}


def _register_relu_add():
    """Custom DVE op: out = relu(in0 + in1), one Vector instruction for the
    bias+relu+fp16-cast of the transposed q/k tiles."""
    if "op" in _RELU_ADD:
        return _RELU_ADD["op"]
    import concourse.dve_ops as dve_ops
    from concourse.dve_spec import Spec, Src0, Src1, relu, lower, _has_src1
    from concourse.dve_uop import DveOpSpec

    def _ref(in0, in1, c0, c1, c2):
        s = in0.astype(np.float32) + in1.astype(np.float32)
        return np.maximum(
            np.nan_to_num(s, nan=0.0, posinf=np.inf, neginf=-np.inf), 0)

    name = "RELU_ADD_ANT"
    if name in dve_ops._SUB_OPCODE_FOR_NAME:
        op = next(o for o in dve_ops.OPS if o.name == name)
        _RELU_ADD["op"] = op
        return op
    spec = Spec(body=relu(Src0 + Src1), reference=_ref)
    row = max(dve_ops._SUB_OPCODE_FOR_NAME.values()) + 1
    assert row < 0x20
    shas = {}
    for ver in ("v3",):
        tmp = DveOpSpec(name=name, opcode=row, uops=lower(spec, ver=ver),
                        rd1_en=_has_src1(spec))
        shas[ver] = tmp.sha(ver)
    op = dve_ops.DveOp(name, spec, subdim=False, uops_sha=shas)
    dve_ops.OPS.append(op)
    dve_ops._SUB_OPCODE_FOR_NAME[name] = row
    dve_ops.CUSTOM_DVE_SPECS[name] = spec
    _RELU_ADD["op"] = op
    return op


LAST_EXEC_NS = None
LAST_RESULTS = None

_CACHE = {}

_AXON_SO = "/opt/axon/libaxon_pjrt.so"


def _setup_profiling():
    """Make run_bass_kernel_spmd(trace=True) work in this container: inject
    the antenv.axon_hooks module (absent in the agent image) with the
    ctypes NTFF hook, and keep profile artifacts local."""
    import sys
    import types

    import concourse.bass_utils as bu

    if not getattr(bu, "_local_artifacts", False):
        bu.upload_artifacts = lambda tmpdir: tmpdir
        bu._local_artifacts = True

    if "antenv.axon_hooks" not in sys.modules:
        mod = types.ModuleType("antenv.axon_hooks")
        mod._hook = None

        def set_axon_ntff_profile_hook(h):
            mod._hook = h

        def get_axon_ntff_profile_hook():
            return mod._hook

        mod.set_axon_ntff_profile_hook = set_axon_ntff_profile_hook
        mod.get_axon_ntff_profile_hook = get_axon_ntff_profile_hook
        sys.modules["antenv.axon_hooks"] = mod
        import antenv

        antenv.axon_hooks = mod
        try:
            from trn_agent_boot.trn_boot import _ntff_profile_via_ctypes

            mod._hook = _ntff_profile_via_ctypes(_AXON_SO)
        except Exception as e:  # profiling degrades, run still works
            print(f"NTFF hook install failed: {e}")


def _build_bass():
    relu_add = _register_relu_add()
    nc = bacc.Bacc("TRN2", target_bir_lowering=False)

    pc_d = nc.declare_dram_parameter("pc", [C, N], F32R, isOutput=False)
    xt_d = nc.declare_dram_parameter("xt5", [C, NCH * 5], F16, isOutput=False)
    wqk_d = nc.declare_dram_parameter("wqk", [C, 2 * C], F32R, isOutput=False)
    wv_d = nc.declare_dram_parameter("wvT", [C, C], F32R, isOutput=False)
    brep_d = nc.declare_dram_parameter("brep", [C, 2 * C], F32, isOutput=False)
    bv_d = nc.declare_dram_parameter("bv", [C, 1], F32, isOutput=False)
    wgt_d = nc.declare_dram_parameter("wgt", [5, C], F32R, isOutput=False)
    idh_d = nc.declare_dram_parameter("idf16", [C, C], F16, isOutput=False)
    out_d = nc.declare_dram_parameter("out", [C, N], F32, isOutput=True)

    with tile.TileContext(nc) as tc:
        with (
            tc.tile_pool(name="persist", bufs=1) as persist,
            tc.tile_pool(name="consts", bufs=1) as consts,
            tc.tile_pool(name="attn_ps", bufs=1, space="PSUM") as attn_pool,
        ):
            # constants
            wqk_s = consts.tile([C, 2 * C], F32R)
            nc.sync.dma_start(out=wqk_s, in_=wqk_d[:, :])
            wv_s = consts.tile([C, C], F32R)
            nc.sync.dma_start(out=wv_s, in_=wv_d[:, :])
            brep_s = consts.tile([C, 2 * C], F32)
            nc.sync.dma_start(out=brep_s, in_=brep_d[:, :])
            bv_s = consts.tile([C, 1], F32)
            nc.sync.dma_start(out=bv_s, in_=bv_d[:, :])
            wgt_s = consts.tile([5, C], F32R)
            nc.sync.dma_start(out=wgt_s, in_=wgt_d[:, :])
            idh_s = consts.tile([C, C], F16)
            nc.sync.dma_start(out=idh_s, in_=idh_d[:, :])
            xt_all = consts.tile([C, NCH * 5], F16)
            nc.sync.dma_start(out=xt_all, in_=xt_d[:, :])
            if PE_BIAS_NUM > 0:
                ones_s = consts.tile([1, CH], F32R)
                nc.vector.memset(ones_s, 1.0)
                brow_s = consts.tile([1, 2 * C], F32R)
                nc.sync.dma_start(out=brow_s, in_=brep_d[0:1, :].bitcast(F32R))

            # v, resident in SBUF for the whole kernel (fp16, ~39KB/partition)
            v_s = persist.tile([C, N], F16)

            # attn logits accumulator [c, d] and the transposed rank-5
            # graph-context accumulator [j, c]; separate PSUM banks
            attn_ps = attn_pool.tile([C, C], F32, tag="attn")
            s2t_acc = attn_pool.tile([5, C], F32, tag="s2t")

            gchunk = 0
            nchunks = sum((ts + CH - 1) // CH for _, ts in TILES)

            with (
                tc.tile_pool(name="pc", bufs=3) as pc_pool,
                tc.tile_pool(name="comb", bufs=3) as comb_pool,
                tc.tile_pool(name="qk_ps", bufs=3, space="PSUM") as qk_ps_pool,
                tc.tile_pool(name="v_ps", bufs=2, space="PSUM") as v_ps_pool,
            ):
                for n0, ts in TILES:
                    pc_t = pc_pool.tile([C, TS], F32R, tag="pc")
                    nc.sync.dma_start(out=pc_t[:, :ts], in_=pc_d[:, n0:n0 + ts])

                    # v tile, natural layout
                    v_ps = v_ps_pool.tile([C, TS], F32, tag="vps")
                    nc.tensor.matmul(
                        v_ps[:, :ts], lhsT=wv_s, rhs=pc_t[:, :ts],
                        start=True, stop=True,
                    )
                    nc.scalar.activation(
                        v_s[:, n0:n0 + ts], v_ps[:, :ts],
                        mybir.ActivationFunctionType.Relu,
                        bias=bv_s, scale=1.0,
                    )

                    for c0 in range(0, ts, CH):
                        nt = min(CH, ts - c0)
                        pe_bias = (gchunk % PE_BIAS_DEN) < PE_BIAS_NUM

                        qk_ps = qk_ps_pool.tile([CH, 2 * C], F32, tag="qkps")
                        nc.tensor.matmul(
                            qk_ps[:nt, :],
                            lhsT=pc_t[:, c0:c0 + nt],
                            rhs=wqk_s,
                            start=True, stop=not pe_bias,
                        )
                        comb = comb_pool.tile([CH, 2 * C], F16, tag="comb")
                        if pe_bias:
                            nc.tensor.matmul(
                                qk_ps[:nt, :],
                                lhsT=ones_s[:, :nt],
                                rhs=brow_s,
                                start=False, stop=True,
                            )
                            nc.scalar.activation(
                                comb[:nt, :], qk_ps[:nt, :],
                                mybir.ActivationFunctionType.Relu,
                                bias=0.0, scale=1.0,
                            )
                        else:
                            nc.vector._custom_dve(
                                relu_add,
                                out=comb[:nt, :],
                                in0=qk_ps[:nt, :],
                                in1=brep_s[:nt, :],
                            )
                        nc.tensor.matmul(
                            attn_ps,
                            lhsT=comb[:nt, 0:C],
                            rhs=comb[:nt, C:2 * C],
                            start=(gchunk == 0), stop=False,
                            skip_group_check=True,
                        )
                        nc.tensor.matmul(
                            s2t_acc,
                            lhsT=xt_all[:nt, gchunk * 5:(gchunk + 1) * 5],
                            rhs=comb[:nt, 0:C],
                            start=(gchunk == 0), stop=(gchunk == nchunks - 1),
                            skip_group_check=True,
                        )
                        gchunk += 1
                assert gchunk == nchunks

            # ---- epilogue: graph-context correction + softmax ----
            with (
                tc.tile_pool(name="epi", bufs=1) as epi,
                tc.tile_pool(name="epi_ps", bufs=1, space="PSUM") as epi_ps,
            ):
                s2t_s = epi.tile([5, C], F32R)
                nc.scalar.copy(s2t_s, s2t_acc)
                nc.tensor.matmul(
                    attn_ps,
                    lhsT=s2t_s,
                    rhs=wgt_s,
                    start=False, stop=True,
                    skip_group_check=True,
                )

                mx = epi.tile([C, 1], F32)
                nc.vector.tensor_reduce(
                    mx, attn_ps,
                    axis=mybir.AxisListType.X, op=mybir.AluOpType.max,
                )
                negmx = epi.tile([C, 1], F32)
                nc.vector.tensor_scalar_mul(negmx, mx, -1.0)
                p_s = epi.tile([C, C], F16)
                denom = epi.tile([C, 1], F32)
                nc.scalar.activation(
                    p_s, attn_ps,
                    mybir.ActivationFunctionType.Exp,
                    bias=negmx, scale=1.0, accum_out=denom,
                )
                inv = persist.tile([C, 1], F32)
                nc.vector.reciprocal(inv, denom)

                attnT_ps = epi_ps.tile([C, C], F16, tag="attnT")
                nc.tensor.transpose(attnT_ps, p_s, idh_s)
                attnT_s = persist.tile([C, C], F16)
                nc.vector.tensor_copy(attnT_s, attnT_ps)

            # ---- phase 2: out = (1/denom) * attn_exp @ v ----
            with (
                tc.tile_pool(name="osb", bufs=3) as o_pool,
                tc.tile_pool(name="o_ps", bufs=2, space="PSUM") as o_ps_pool,
            ):
                for n0, ts in TILES:
                    o_ps = o_ps_pool.tile([C, TS], F32, tag="ops")
                    nc.tensor.matmul(
                        o_ps[:, :ts],
                        lhsT=attnT_s,
                        rhs=v_s[:, n0:n0 + ts],
                        start=True, stop=True,
                    )
                    o_s = o_pool.tile([C, TS], F32, tag="osb")
                    nc.scalar.activation(
                        o_s[:, :ts], o_ps[:, :ts],
                        mybir.ActivationFunctionType.Copy,
                        bias=0.0, scale=inv,
                    )
                    nc.sync.dma_start(out=out_d[:, n0:n0 + ts], in_=o_s[:, :ts])

    nc.finalize()
    return nc


def _prep_host(inputs):
    """Fold BN + temperature into weights on the host. O(C^2) work only."""
    f = np.float32
    pc = np.ascontiguousarray(np.asarray(inputs["PointCN1"], f)[..., 0])  # [B,C,N]
    x = np.asarray(inputs["x"], f)[..., 0]                                # [B,4,N]

    def fold(W, g, b, m, v, temp=1.0):
        # y*scale + (b - m*scale), then /temp, all folded into W and shift
        s = (np.asarray(g, f) / np.sqrt(np.asarray(v, f) + EPS)) / temp
        Wp = np.asarray(W, f) * s[:, None]
        sh = (np.asarray(b, f) / temp) - np.asarray(m, f) * s
        return Wp, sh

    temp = np.sqrt(np.float32(C))
    Wq, sq = fold(inputs["Wq"], inputs["gq"], inputs["bq"], inputs["mq"],
                  inputs["vq"], temp)
    Wk, sk = fold(inputs["Wk"], inputs["gk"], inputs["bk"], inputs["mk"],
                  inputs["vk"])
    Wv, sv = fold(inputs["Wv"], inputs["gv"], inputs["bv"], inputs["mv"],
                  inputs["vv"])

    wqk = np.ascontiguousarray(np.concatenate([Wq.T, Wk.T], axis=1))  # [C,256]
    wvT = np.ascontiguousarray(np.asarray(Wv, f).T)                   # [C,C]
    brep = np.ascontiguousarray(
        np.tile(np.concatenate([sq, sk])[None, :], (C, 1)).astype(f))  # [C,256]
    bv = np.ascontiguousarray(sv[:, None])                            # [C,1]

    Wg1 = np.asarray(inputs["Wg1"], f)  # [C,2]
    Wg2 = np.asarray(inputs["Wg2"], f)  # [C,2]
    bg = np.asarray(inputs["bg1"], f) + np.asarray(inputs["bg2"], f)  # [C]
    wgext = np.concatenate([Wg1, Wg2, bg[:, None]], axis=1)           # [C,5]
    wgt = np.ascontiguousarray(wgext.T)                               # [5,C]

    # xt5 packed partition-major: xt5p[b, p, g*5+j] = [x^T | 1][g*128+p, j]
    npad = NCH * CH
    xt5 = np.zeros((B, npad, 5), np.float16)
    for b in range(B):
        xt5[b, :N, 0:4] = x[b].T.astype(np.float16)
        xt5[b, :N, 4] = np.float16(1.0)
    xt5p = np.ascontiguousarray(
        xt5.reshape(B, NCH, CH, 5).transpose(0, 2, 1, 3).reshape(B, CH, NCH * 5))

    idf16 = np.eye(C, dtype=np.float16)

    in_maps = []
    for b in range(B):
        in_maps.append({
            "pc": pc[b],
            "xt5": xt5p[b],
            "wqk": wqk,
            "wvT": wvT,
            "brep": brep,
            "bv": bv,
            "wgt": wgt,
            "idf16": idf16,
        })
    return in_maps


def kernel(profile=False, **inputs):
    global LAST_EXEC_NS, LAST_RESULTS
    if profile:
        _setup_profiling()
    if "nc" not in _CACHE:
        _CACHE["nc"] = _build_bass()
    nc = _CACHE["nc"]
    in_maps = _prep_host(inputs)
    res = run_bass_kernel_spmd(
        nc, in_maps, core_ids=list(range(B)), trace=bool(profile),
    )
    LAST_EXEC_NS = res.exec_time_ns
    LAST_RESULTS = res
    out = np.stack([res.results[b]["out"] for b in range(B)], axis=0)
    return out[..., None].astype(np.float32)


# revision 14
# speedup vs baseline: 5.5217x; 1.5895x over previous
"""Channel-attention block (q/k/v 1x1-conv+BN+ReLU, attention over channels
contracted over N points, softmax, weighted sum) for B=8, C=128, N=20000.

Sharding: data-parallel over B — one sample per NeuronCore, 8 cores, no
cross-core communication. Inside each core:

  phase 1 (streaming over N in 512-point tiles / 128-point chunks):
    - v = relu(Wv' @ pc + bv')      natural layout [C, n], fp16, kept in SBUF
    - qk^T chunk = pc_chunk^T @ [Wq'^T | Wk'^T]   (one fp32r matmul, N=256)
    - +bias (DVE, replicated-bias tile), relu+cast->fp16 (GPSIMD)
    - attn accumulation (fp16 matmul): [qs·k^T | qs·[x^T|1]] into one
      persistent PSUM bank ([128, 133]); graph-context never materialized.
  epilogue:
    logits = S1 + S2ext @ Wgext^T (rank-5 correction), softmax via
    (-max bias) exp with fused row-sum; normalization deferred to phase 2.
  phase 2:
    out = attn^T-stationary fp16 matmul over v tiles (N=512), PSUM->SBUF
    copy applies the 1/rowsum scale, DMA to DRAM.

All BN/temperature folding is done on the host (weights only, O(C^2)).
"""

import numpy as np

import concourse.bass as bass
import concourse.bacc as bacc
import concourse.tile as tile
from concourse import mybir
from concourse.bass_utils import run_bass_kernel_spmd

F32 = mybir.dt.float32
F32R = mybir.dt.float32r
F16 = mybir.dt.float16

B, C, N = 8, 128, 20000
EPS = 1e-5
TS = 512                     # n-tile size (v matmul free dim / PSUM bank)
CH = 128                     # chunk size (attn contraction partition dim)
TILES = [(i * TS, TS) for i in range(N // TS)]
if N % TS:
    TILES.append((N - N % TS, N % TS))
NCH = sum((ts + CH - 1) // CH for _, ts in TILES)   # 157 chunks

# chunks where the qk bias is applied by a rank-1 PE matmul (ACT does the
# relu straight from PSUM) instead of the fused DVE relu-add. num/den split.
PE_BIAS_NUM = 0
PE_BIAS_DEN = 4

_RELU_ADD = {}


def _register_relu_add():
    """Custom DVE op: out = relu(in0 + in1), one Vector instruction for the
    bias+relu+fp16-cast of the transposed q/k tiles."""
    if "op" in _RELU_ADD:
        return _RELU_ADD["op"]
    import concourse.dve_ops as dve_ops
    from concourse.dve_spec import Spec, Src0, Src1, relu, lower, _has_src1
    from concourse.dve_uop import DveOpSpec

    def _ref(in0, in1, c0, c1, c2):
        s = in0.astype(np.float32) + in1.astype(np.float32)
        return np.maximum(
            np.nan_to_num(s, nan=0.0, posinf=np.inf, neginf=-np.inf), 0)

    name = "RELU_ADD_ANT"
    if name in dve_ops._SUB_OPCODE_FOR_NAME:
        op = next(o for o in dve_ops.OPS if o.name == name)
        _RELU_ADD["op"] = op
        return op
    spec = Spec(body=relu(Src0 + Src1), reference=_ref)
    row = max(dve_ops._SUB_OPCODE_FOR_NAME.values()) + 1
    assert row < 0x20
    shas = {}
    for ver in ("v3",):
        tmp = DveOpSpec(name=name, opcode=row, uops=lower(spec, ver=ver),
                        rd1_en=_has_src1(spec))
        shas[ver] = tmp.sha(ver)
    op = dve_ops.DveOp(name, spec, subdim=False, uops_sha=shas)
    dve_ops.OPS.append(op)
    dve_ops._SUB_OPCODE_FOR_NAME[name] = row
    dve_ops.CUSTOM_DVE_SPECS[name] = spec
    _RELU_ADD["op"] = op
    return op


LAST_EXEC_NS = None
LAST_RESULTS = None

_CACHE = {}

_AXON_SO = "/opt/axon/libaxon_pjrt.so"


def _setup_profiling():
    """Make run_bass_kernel_spmd(trace=True) work in this container: inject
    the antenv.axon_hooks module (absent in the agent image) with the
    ctypes NTFF hook, and keep profile artifacts local."""
    import sys
    import types

    import concourse.bass_utils as bu

    if not getattr(bu, "_local_artifacts", False):
        bu.upload_artifacts = lambda tmpdir: tmpdir
        bu._local_artifacts = True

    if "antenv.axon_hooks" not in sys.modules:
        mod = types.ModuleType("antenv.axon_hooks")
        mod._hook = None

        def set_axon_ntff_profile_hook(h):
            mod._hook = h

        def get_axon_ntff_profile_hook():
            return mod._hook

        mod.set_axon_ntff_profile_hook = set_axon_ntff_profile_hook
        mod.get_axon_ntff_profile_hook = get_axon_ntff_profile_hook
        sys.modules["antenv.axon_hooks"] = mod
        import antenv

        antenv.axon_hooks = mod
        try:
            from trn_agent_boot.trn_boot import _ntff_profile_via_ctypes

            mod._hook = _ntff_profile_via_ctypes(_AXON_SO)
        except Exception as e:  # profiling degrades, run still works
            print(f"NTFF hook install failed: {e}")


def _build_bass():
    relu_add = _register_relu_add()
    nc = bacc.Bacc("TRN2", target_bir_lowering=False)

    pc_d = nc.declare_dram_parameter("pc", [C, N], F16, isOutput=False)
    xt_d = nc.declare_dram_parameter("xt5", [C, NCH * 5], F16, isOutput=False)
    wqk_d = nc.declare_dram_parameter("wqk", [C, 2 * C], F16, isOutput=False)
    wv_d = nc.declare_dram_parameter("wvT", [C, C], F16, isOutput=False)
    brep_d = nc.declare_dram_parameter("brep", [C, 4 * C], F32, isOutput=False)
    bv_d = nc.declare_dram_parameter("bv", [C, 1], F32, isOutput=False)
    wgt_d = nc.declare_dram_parameter("wgt", [5, C], F32R, isOutput=False)
    idh_d = nc.declare_dram_parameter("idf16", [C, C], F16, isOutput=False)
    out_d = nc.declare_dram_parameter("out", [C, N], F32, isOutput=True)

    with tile.TileContext(nc) as tc:
        with (
            tc.tile_pool(name="persist", bufs=1) as persist,
            tc.tile_pool(name="consts", bufs=1) as consts,
            tc.tile_pool(name="attn_ps", bufs=1, space="PSUM") as attn_pool,
        ):
            # constants
            wqk_s = consts.tile([C, 2 * C], F16)
            nc.sync.dma_start(out=wqk_s, in_=wqk_d[:, :])
            wv_s = consts.tile([C, C], F16)
            nc.sync.dma_start(out=wv_s, in_=wv_d[:, :])
            brep_s = consts.tile([C, 4 * C], F32)
            nc.sync.dma_start(out=brep_s, in_=brep_d[:, :])
            bv_s = consts.tile([C, 1], F32)
            nc.sync.dma_start(out=bv_s, in_=bv_d[:, :])
            wgt_s = consts.tile([5, C], F32R)
            nc.sync.dma_start(out=wgt_s, in_=wgt_d[:, :])
            idh_s = consts.tile([C, C], F16)
            nc.sync.dma_start(out=idh_s, in_=idh_d[:, :])
            xt_all = consts.tile([C, NCH * 5], F16)
            nc.sync.dma_start(out=xt_all, in_=xt_d[:, :])

            # v, resident in SBUF for the whole kernel (fp16, ~39KB/partition)
            v_s = persist.tile([C, N], F16)

            # attn logits accumulator [c, d] and the transposed rank-5
            # graph-context accumulator [j, c]; separate PSUM banks
            attn_ps = attn_pool.tile([C, C], F32, tag="attn")
            s2t_acc = attn_pool.tile([5, C], F32, tag="s2t")

            gchunk = 0

            with (
                tc.tile_pool(name="pc", bufs=3) as pc_pool,
                tc.tile_pool(name="comb", bufs=3) as comb_pool,
                tc.tile_pool(name="qk_ps", bufs=3, space="PSUM") as qk_ps_pool,
                tc.tile_pool(name="v_ps", bufs=2, space="PSUM") as v_ps_pool,
            ):
                for n0, ts in TILES:
                    pc_t = pc_pool.tile([C, TS], F16, tag="pc")
                    nc.sync.dma_start(out=pc_t[:, :ts], in_=pc_d[:, n0:n0 + ts])

                    # v tile, natural layout
                    v_ps = v_ps_pool.tile([C, TS], F32, tag="vps")
                    nc.tensor.matmul(
                        v_ps[:, :ts], lhsT=wv_s, rhs=pc_t[:, :ts],
                        start=True, stop=True,
                    )
                    nc.scalar.activation(
                        v_s[:, n0:n0 + ts], v_ps[:, :ts],
                        mybir.ActivationFunctionType.Relu,
                        bias=bv_s, scale=1.0,
                    )

                    # chunk pairs: two 128-pt chunks share one PSUM bank and
                    # one fused DVE relu(x+bias) op
                    pair = [(c0, min(CH, ts - c0)) for c0 in range(0, ts, CH)]
                    for p0 in range(0, len(pair), 2):
                        grp = pair[p0:p0 + 2]
                        qk_ps = qk_ps_pool.tile([CH, 4 * C], F32, tag="qkps")
                        comb = comb_pool.tile([CH, 4 * C], F16, tag="comb")
                        maxnt = max(nt for _, nt in grp)
                        for j, (c0, nt) in enumerate(grp):
                            nc.tensor.matmul(
                                qk_ps[:nt, j * 2 * C:(j + 1) * 2 * C],
                                lhsT=pc_t[:, c0:c0 + nt],
                                rhs=wqk_s,
                                start=(j == 0), stop=(j == len(grp) - 1),
                                skip_group_check=True,
                            )
                        width = len(grp) * 2 * C
                        nc.vector._custom_dve(
                            relu_add,
                            out=comb[:maxnt, 0:width],
                            in0=qk_ps[:maxnt, 0:width],
                            in1=brep_s[:maxnt, 0:width],
                        )
                        for j, (c0, nt) in enumerate(grp):
                            o = j * 2 * C
                            nc.tensor.matmul(
                                attn_ps,
                                lhsT=comb[:nt, o:o + C],
                                rhs=comb[:nt, o + C:o + 2 * C],
                                start=(gchunk == 0), stop=False,
                                skip_group_check=True,
                            )
                            nc.tensor.matmul(
                                s2t_acc,
                                lhsT=xt_all[:nt, gchunk * 5:(gchunk + 1) * 5],
                                rhs=comb[:nt, o:o + C],
                                start=(gchunk == 0), stop=(gchunk == NCH - 1),
                                skip_group_check=True,
                            )
                            gchunk += 1
                assert gchunk == NCH

            # ---- epilogue: graph-context correction + softmax ----
            with (
                tc.tile_pool(name="epi", bufs=1) as epi,
                tc.tile_pool(name="epi_ps", bufs=1, space="PSUM") as epi_ps,
            ):
                s2t_s = epi.tile([5, C], F32R)
                nc.scalar.copy(s2t_s, s2t_acc)
                nc.tensor.matmul(
                    attn_ps,
                    lhsT=s2t_s,
                    rhs=wgt_s,
                    start=False, stop=True,
                    skip_group_check=True,
                )

                mx = epi.tile([C, 1], F32)
                nc.vector.tensor_reduce(
                    mx, attn_ps,
                    axis=mybir.AxisListType.X, op=mybir.AluOpType.max,
                )
                negmx = epi.tile([C, 1], F32)
                nc.vector.tensor_scalar_mul(negmx, mx, -1.0)
                p_s = epi.tile([C, C], F16)
                denom = epi.tile([C, 1], F32)
                nc.scalar.activation(
                    p_s, attn_ps,
                    mybir.ActivationFunctionType.Exp,
                    bias=negmx, scale=1.0, accum_out=denom,
                )
                inv = persist.tile([C, 1], F32)
                nc.vector.reciprocal(inv, denom)

                attnT_ps = epi_ps.tile([C, C], F16, tag="attnT")
                nc.tensor.transpose(attnT_ps, p_s, idh_s)
                attnT_s = persist.tile([C, C], F16)
                nc.vector.tensor_copy(attnT_s, attnT_ps)

            # ---- phase 2: out = (1/denom) * attn_exp @ v ----
            P2 = 2 * TS
            p2tiles = [(i * P2, min(P2, N - i * P2))
                       for i in range((N + P2 - 1) // P2)]
            with (
                tc.tile_pool(name="osb", bufs=3) as o_pool,
                tc.tile_pool(name="o_ps", bufs=2, space="PSUM") as o_ps_pool,
            ):
                for n0, ts in p2tiles:
                    o_ps = o_ps_pool.tile([C, P2], F32, tag="ops")
                    for s0 in range(0, ts, TS):
                        w = min(TS, ts - s0)
                        nc.tensor.matmul(
                            o_ps[:, s0:s0 + w],
                            lhsT=attnT_s,
                            rhs=v_s[:, n0 + s0:n0 + s0 + w],
                            start=True, stop=True,
                        )
                    o_s = o_pool.tile([C, P2], F32, tag="osb")
                    nc.scalar.activation(
                        o_s[:, :ts], o_ps[:, :ts],
                        mybir.ActivationFunctionType.Copy,
                        bias=0.0, scale=inv,
                    )
                    nc.sync.dma_start(out=out_d[:, n0:n0 + ts], in_=o_s[:, :ts])

    nc.finalize()
    return nc


def _prep_host(inputs):
    """Fold BN + temperature into weights on the host. O(C^2) work only."""
    f = np.float32
    pc = np.asarray(inputs["PointCN1"], f)[..., 0]                    # [B,C,N]
    pc16 = np.ascontiguousarray(pc.astype(np.float16))
    x = np.asarray(inputs["x"], f)[..., 0]                                # [B,4,N]

    def fold(W, g, b, m, v, temp=1.0):
        # y*scale + (b - m*scale), then /temp, all folded into W and shift
        s = (np.asarray(g, f) / np.sqrt(np.asarray(v, f) + EPS)) / temp
        Wp = np.asarray(W, f) * s[:, None]
        sh = (np.asarray(b, f) / temp) - np.asarray(m, f) * s
        return Wp, sh

    temp = np.sqrt(np.float32(C))
    Wq, sq = fold(inputs["Wq"], inputs["gq"], inputs["bq"], inputs["mq"],
                  inputs["vq"], temp)
    Wk, sk = fold(inputs["Wk"], inputs["gk"], inputs["bk"], inputs["mk"],
                  inputs["vk"])
    Wv, sv = fold(inputs["Wv"], inputs["gv"], inputs["bv"], inputs["mv"],
                  inputs["vv"])

    wqk = np.ascontiguousarray(
        np.concatenate([Wq.T, Wk.T], axis=1).astype(np.float16))      # [C,256]
    wvT = np.ascontiguousarray(np.asarray(Wv, f).T.astype(np.float16))
    shifts = np.concatenate([sq, sk])
    brep = np.ascontiguousarray(
        np.tile(np.concatenate([shifts, shifts])[None, :], (C, 1)).astype(f))
    bv = np.ascontiguousarray(sv[:, None])                            # [C,1]

    Wg1 = np.asarray(inputs["Wg1"], f)  # [C,2]
    Wg2 = np.asarray(inputs["Wg2"], f)  # [C,2]
    bg = np.asarray(inputs["bg1"], f) + np.asarray(inputs["bg2"], f)  # [C]
    wgext = np.concatenate([Wg1, Wg2, bg[:, None]], axis=1)           # [C,5]
    wgt = np.ascontiguousarray(wgext.T)                               # [5,C]

    # xt5 packed partition-major: xt5p[b, p, g*5+j] = [x^T | 1][g*128+p, j]
    npad = NCH * CH
    xt5 = np.zeros((B, npad, 5), np.float16)
    for b in range(B):
        xt5[b, :N, 0:4] = x[b].T.astype(np.float16)
        xt5[b, :N, 4] = np.float16(1.0)
    xt5p = np.ascontiguousarray(
        xt5.reshape(B, NCH, CH, 5).transpose(0, 2, 1, 3).reshape(B, CH, NCH * 5))

    idf16 = np.eye(C, dtype=np.float16)

    in_maps = []
    for b in range(B):
        in_maps.append({
            "pc": pc16[b],
            "xt5": xt5p[b],
            "wqk": wqk,
            "wvT": wvT,
            "brep": brep,
            "bv": bv,
            "wgt": wgt,
            "idf16": idf16,
        })
    return in_maps


def kernel(profile=False, **inputs):
    global LAST_EXEC_NS, LAST_RESULTS
    if profile:
        _setup_profiling()
    if "nc" not in _CACHE:
        _CACHE["nc"] = _build_bass()
    nc = _CACHE["nc"]
    in_maps = _prep_host(inputs)
    res = run_bass_kernel_spmd(
        nc, in_maps, core_ids=list(range(B)), trace=bool(profile),
    )
    LAST_EXEC_NS = res.exec_time_ns
    LAST_RESULTS = res
    out = np.stack([res.results[b]["out"] for b in range(B)], axis=0)
    return out[..., None].astype(np.float32)
